# revision 1
# baseline (speedup 1.0000x reference)
"""Trainium2 Bass kernel v2 for LowRankOrthogonalProjection.

    out = target + (source - target) @ W @ W.T        (W: [D, R], R=16)

Key changes vs baseline:
  * Host pre-transposes/packs src/tgt/out into per-slab tiles with the
    D-chunk dim on partitions, eliminating all PE transposes.
  * source travels as fp8e4 (8.4 MB/core instead of 16.8); upcast either
    inline in a SWDGE cast-DMA or by the sub op.  Projection through the
    rank-16 subspace attenuates the quantization error by sqrt(R/D)=1/16.
  * target and out are bf16 (out upcast to f32 on host).
  * element-wise ops arranged for DVE 2x mode where possible; PSUM->SBUF
    moves on the otherwise idle ACT engine.

Per-core HBM traffic: 8.4 + 16.8 + 16.8 = 42 MB vs 84 MB baseline.

Layout (per core, rpc=2048 rows in NQ slabs of QR=2048/NQ rows):
  srcp[q*128+p, c*QR+r] = src[q*QR+r, c*128+p]      (fp8) [NQ*128, 32*QR]
  tgtp / outp: same indexing, bf16.
  wc[p, c*16+j]  = W[c*128+p, j]                          [128, 512]
  wt[j, d]       = W[d, j]                                [16, 4096]
"""

import numpy as np
import ml_dtypes

B, S, D, R = 4, 4096, 4096, 16
N_CORES = 8
ROWS = B * S                 # 16384
RPC = ROWS // N_CORES        # 2048 rows per core
P = 128
DCH = D // P                 # 32 D-chunks
NQ = 4                       # default slabs per core
WSCALE = 32.0                # fp8 weight scale for DoubleRow mode
_NC_CACHE = {}

# default build config (kernel() uses this; bench can override)
CFG = dict(nq=4, src_cast_dma=True, sub_engine="dve", out_dma="sync",
           copy_split=0, two_pass_a=False, pb_group=2, tgt_split=1,
           tgt_bufs=6, out_split=2, src_split=2, src_bufs=3, out_bufs=2,
           tgt_tiles=2)


def build_nc(reps=1, nq=4, src_cast_dma=True, sub_engine="dve",
             out_dma="sync", copy_split=0, two_pass_a=False, pb_group=1,
             tgt_split=1, tgt_bufs=2, out_split=2, src_split=2, src_bufs=3,
             out_bufs=2, tgt_tiles=1):
    import concourse.bass as bass
    import concourse.mybir as mybir
    import concourse.tile as tile

    bf16 = mybir.dt.bfloat16
    f32 = mybir.dt.float32
    f8 = mybir.dt.float8e4

    QR = RPC // nq           # rows per slab
    QW = DCH * QR            # packed free width

    nc = bass.Bass("TRN2", target_bir_lowering=False)

    srcp = nc.dram_tensor("srcp", [nq * P, QW], f8, kind="ExternalInput")
    tgtp = nc.dram_tensor("tgtp", [nq * P, QW], bf16, kind="ExternalInput")
    wc = nc.dram_tensor("wc", [P, DCH * R], bf16, kind="ExternalInput")
    wcn = nc.dram_tensor("wcn", [P, DCH * R], bf16, kind="ExternalInput")
    # DoubleRow stationary operand: wc8[p, c2*32 + b*16 + j] = WSCALE*W[c2*256+2p+b, j]
    wc8 = nc.dram_tensor("wc8", [P, DCH * R], f8, kind="ExternalInput")
    wt = nc.dram_tensor("wt", [R, D], bf16, kind="ExternalInput")
    outp = nc.dram_tensor("outp", [nq * P, QW], bf16, kind="ExternalOutput")

    HW = QW // 2             # half-slab free width
    sub_eng = dict(dve="vector", pool="gpsimd")[sub_engine]
    out_eng = dict(sync="sync", scalar="scalar")[out_dma]

    with tile.TileContext(nc) as tc:
        with (
            tc.tile_pool(name="const", bufs=1) as cpool,
            tc.tile_pool(name="tgtp_", bufs=tgt_bufs) as tgt_pool,
            tc.tile_pool(name="srcp_", bufs=src_bufs) as src_pool,
            tc.tile_pool(name="diffp", bufs=3) as diff_pool,
            tc.tile_pool(name="tmtp", bufs=2) as tmt_pool,
            tc.tile_pool(name="corrp", bufs=4) as corr_pool,
            tc.tile_pool(name="outp_", bufs=out_bufs) as out_pool,
            tc.tile_pool(name="ps_t", bufs=2, space="PSUM") as ps_t,
            tc.tile_pool(
                name="ps_o", bufs=(4 if pb_group == 1 else 3), space="PSUM"
            ) as ps_o,
        ):
            wc_sb = cpool.tile([P, DCH * R], bf16)
            nc.sync.dma_start(wc_sb, wc[:, :])
            wt_sb = cpool.tile([R, D], bf16)
            nc.sync.dma_start(wt_sb, wt[:, :])
            if two_pass_a:
                wcn_sb = cpool.tile([P, DCH * R], bf16)
                nc.sync.dma_start(wcn_sb, wcn[:, :])
            if two_pass_a == "dr":
                wc8_sb = cpool.tile([P, DCH * R], f8)
                nc.sync.dma_start(wc8_sb, wc8[:, :])

            for qi in range(nq * reps):
                q = qi % nq
                qs = q * P

                if tgt_tiles == 2:
                    # two independently-released half tiles: earlier buffer
                    # turnaround for the next quarter's tgt DMA
                    tgt_halves = []
                    for th in range(2):
                        t = tgt_pool.tile([P, HW], bf16, tag="tgt")
                        nc.sync.dma_start(
                            t, tgtp[qs : qs + P, th * HW : (th + 1) * HW]
                        )
                        tgt_halves.append(t)

                    class _TgtView:
                        def __getitem__(self, idx):
                            _, sl = idx
                            a, b = sl.start, sl.stop
                            h = a // HW
                            assert b <= (h + 1) * HW
                            return tgt_halves[h][:, a - h * HW : b - h * HW]

                    tgt_sb = _TgtView()
                else:
                    tgt_sb = tgt_pool.tile([P, QW], bf16, tag="tgt")
                    tw = QW // tgt_split
                    for ts in range(tgt_split):
                        nc.sync.dma_start(
                            tgt_sb[:, ts * tw : (ts + 1) * tw],
                            tgtp[qs : qs + P, ts * tw : (ts + 1) * tw],
                        )

                if src_split == 1:
                    sdt = bf16 if src_cast_dma else f8
                    seng = nc.gpsimd if src_cast_dma else nc.sync
                    s = src_pool.tile([P, QW], sdt, tag="src")
                    seng.dma_start(s, srcp[qs : qs + P, :])
                    src_sb = [s[:, 0:HW], s[:, HW:QW]]
                else:
                    src_sb = []
                    for h in range(2):
                        if src_cast_dma:
                            # fp8 on the wire, bf16 in SBUF (SWDGE casts)
                            s = src_pool.tile([P, HW], bf16, tag="src")
                            nc.gpsimd.dma_start(
                                s, srcp[qs : qs + P, h * HW : (h + 1) * HW]
                            )
                        else:
                            # fp8 in SBUF; PE rhs / the sub upconverts
                            s = src_pool.tile([P, HW], f8, tag="src")
                            nc.sync.dma_start(
                                s, srcp[qs : qs + P, h * HW : (h + 1) * HW]
                            )
                        src_sb.append(s)

                tT_ps = ps_t.tile([R, QR], f32, tag="tT")
                if two_pass_a == "dr":
                    # src pass in fp8 DoubleRow (K=256 per matmul, scaled
                    # weights), tgt pass in bf16 with matching -WSCALE*W.
                    import concourse.bass as bass

                    def ap3(t2d, ko, dim):
                        a = [list(x) for x in t2d.ap]
                        return bass.AP(
                            t2d.tensor, t2d.offset,
                            [a[0], [dim, ko], [1, dim]],
                        )

                    DC2 = DCH // 2
                    first = True
                    for h in range(2):
                        for j in range(DC2 // 2):
                            c2 = h * (DC2 // 2) + j
                            nc.tensor.matmul(
                                tT_ps,
                                ap3(wc8_sb[:, c2 * 2 * R : (c2 + 1) * 2 * R], 2, R),
                                ap3(src_sb[h][:, j * 2 * QR : (j + 1) * 2 * QR], 2, QR),
                                start=first,
                                stop=False,
                                perf_mode=mybir.MatmulPerfMode.DoubleRow,
                            )
                            first = False
                        for jc in range(DCH // 2):
                            c = h * (DCH // 2) + jc
                            nc.tensor.matmul(
                                tT_ps,
                                wcn_sb[:, c * R : (c + 1) * R],
                                tgt_sb[:, c * QR : (c + 1) * QR],
                                start=False,
                                stop=(c == DCH - 1),
                            )
                elif two_pass_a:
                    # t = src@W - tgt@W via PSUM accumulation, no sub op
                    for c in range(DCH):
                        h, off = divmod(c * QR, HW)
                        nc.tensor.matmul(
                            tT_ps,
                            wc_sb[:, c * R : (c + 1) * R],
                            src_sb[h][:, off : off + QR],
                            start=(c == 0),
                            stop=False,
                        )
                        nc.tensor.matmul(
                            tT_ps,
                            wcn_sb[:, c * R : (c + 1) * R],
                            tgt_sb[:, c * QR : (c + 1) * QR],
                            start=False,
                            stop=(c == DCH - 1),
                        )
                else:
                    gw = 4 * QR          # sub granularity: 4 chunks
                    for g in range(8):
                        d_sb = diff_pool.tile([P, gw], bf16, tag="diff")
                        h, off = divmod(g * gw, HW)
                        getattr(nc, sub_eng).tensor_sub(
                            d_sb,
                            src_sb[h][:, off : off + gw],
                            tgt_sb[:, g * gw : (g + 1) * gw],
                        )
                        for j in range(4):
                            c = g * 4 + j
                            nc.tensor.matmul(
                                tT_ps,
                                wc_sb[:, c * R : (c + 1) * R],
                                d_sb[:, j * QR : (j + 1) * QR],
                                start=(c == 0),
                                stop=(c == DCH - 1),
                            )

                tT_sb = tmt_pool.tile([R, QR], bf16, tag="tTs")
                if two_pass_a == "dr":
                    nc.scalar.mul(tT_sb, tT_ps, 1.0 / WSCALE)
                else:
                    nc.scalar.copy(tT_sb, tT_ps)

                PB = pb_group          # D-chunks per psum tile / ACT copy
                OW = QW // out_split   # free width per out tile/DMA
                out_sb = None
                ob = 0
                for h in range(2):
                    if out_sb is None:
                        out_sb = out_pool.tile([P, OW], bf16, tag="out")
                        ob = h * (DCH // 2)   # base chunk of this out tile
                    for j in range(DCH // 2 // PB):
                        o_ps = ps_o.tile([P, PB * QR], f32, tag="ops")
                        for b in range(PB):
                            c = h * (DCH // 2) + j * PB + b
                            nc.tensor.matmul(
                                o_ps[:, b * QR : (b + 1) * QR],
                                wt_sb[:, c * P : (c + 1) * P],
                                tT_sb,
                                start=True,
                                stop=True,
                            )
                        c0 = h * (DCH // 2) + j * PB
                        corr = corr_pool.tile([P, PB * QR], bf16, tag="corr")
                        if copy_split and (j % copy_split == copy_split - 1):
                            nc.vector.tensor_copy(corr, o_ps)
                        else:
                            nc.scalar.copy(corr, o_ps)
                        nc.vector.tensor_add(
                            out_sb[:, (c0 - ob) * QR : (c0 - ob + PB) * QR],
                            corr,
                            tgt_sb[:, c0 * QR : (c0 + PB) * QR],
                        )
                    if (h + 1) % (2 // out_split) == 0:
                        getattr(nc, out_eng).dma_start(
                            outp[qs : qs + P, ob * QR : ob * QR + OW], out_sb
                        )
                        out_sb = None

    return nc


def split_waits(nc, limit=1):
    """Walrus encodes at most one semaphore wait per instruction; hoist
    extras onto standalone EventSemaphore instructions."""
    import concourse.mybir as mybir

    nsplit = 0
    for fn in nc.m.functions:
        for blk in fn.blocks:
            new = []
            for ins in blk.instructions:
                si = ins.sync_info
                waits = list(si.on_wait) if si is not None and si.on_wait else []
                if len(waits) > limit:
                    for k, w in enumerate(waits[:-limit]):
                        es = mybir.InstEventSemaphore(
                            name=f"{ins.name}-hw{k}",
                            engine=ins.engine,
                            sync_info=mybir.SyncInfo(on_wait=[w], on_update=[]),
                        )
                        new.append(es)
                        nsplit += 1
                    ins.sync_info = mybir.SyncInfo(
                        on_wait=waits[-limit:],
                        on_update=list(si.on_update or []),
                    )
                new.append(ins)
            blk.instructions[:] = new
    return nsplit


def _get_nc(reps=1, raw=False, **kw):
    cfg = dict(CFG)
    cfg.update(kw)
    key = (reps, raw, tuple(sorted(cfg.items())))
    if key not in _NC_CACHE:
        nc = build_nc(reps, **cfg)
        nc.finalize()
        if not raw:
            split_waits(nc)
        _NC_CACHE[key] = nc
    return _NC_CACHE[key]


def _pack(x2, dtype, nq):
    """[16384, 4096] row-major -> per-core [nq*128, 32*QR] slab-packed."""
    QR = RPC // nq
    xq = np.asarray(x2).astype(dtype)
    xp = (
        xq.reshape(N_CORES, nq, QR, DCH, P)
        .transpose(0, 1, 4, 3, 2)
        .reshape(N_CORES, nq * P, DCH * QR)
    )
    return np.ascontiguousarray(xp)


def _pack_dr(x2, dtype, nq):
    """DoubleRow src pack: [q*128+p, c2*2*QR + b*QR + r] = x[q*QR+r, c2*256+2p+b]."""
    QR = RPC // nq
    DC2 = DCH // 2
    xq = np.asarray(x2).astype(dtype)
    xp = (
        xq.reshape(N_CORES, nq, QR, DC2, P, 2)     # (core, q, r, c2, p, b)
        .transpose(0, 1, 4, 3, 5, 2)               # (core, q, p, c2, b, r)
        .reshape(N_CORES, nq * P, DCH * QR)
    )
    return np.ascontiguousarray(xp)


def make_host_inputs(source, target, weight, nq=None, dr=None):
    import concourse.mybir as mybir

    nq = nq or CFG["nq"]
    if dr is None:
        dr = CFG.get("two_pass_a") == "dr"
    f8dt = mybir.dt.np(mybir.dt.float8e4)
    bf = ml_dtypes.bfloat16

    if dr:
        srcp = _pack_dr(source.reshape(ROWS, D), f8dt, nq)
        wsc = WSCALE
    else:
        srcp = _pack(source.reshape(ROWS, D), f8dt, nq)
        wsc = 1.0
    tgtp = _pack(target.reshape(ROWS, D), bf, nq)
    wc = np.ascontiguousarray(
        weight.reshape(DCH, P, R).transpose(1, 0, 2).reshape(P, DCH * R)
    ).astype(bf)
    wcn = np.ascontiguousarray(
        (-wsc * weight).reshape(DCH, P, R).transpose(1, 0, 2).reshape(P, DCH * R)
    ).astype(bf)
    # wc8[p, c2*32 + b*16 + j] = WSCALE*W[c2*256 + 2p + b, j]
    wc8 = np.ascontiguousarray(
        np.clip(WSCALE * weight, -240, 240)
        .reshape(DCH // 2, P, 2, R)
        .transpose(1, 0, 2, 3)
        .reshape(P, DCH * R)
    ).astype(f8dt)
    wt = np.ascontiguousarray(weight.T).astype(bf)

    return [
        {"srcp": srcp[c], "tgtp": tgtp[c], "wc": wc, "wcn": wcn,
         "wc8": wc8, "wt": wt}
        for c in range(N_CORES)
    ]


def unpack_output(res_list, nq=None):
    """per-core [nq*128, 32*QR] bf16 -> [B, S, D] f32."""
    nq = nq or CFG["nq"]
    QR = RPC // nq
    outp = np.stack([r["outp"] for r in res_list])
    out = (
        outp.reshape(N_CORES, nq, P, DCH, QR)
        .transpose(0, 1, 4, 3, 2)
        .astype(np.float32)
        .reshape(B, S, D)
    )
    return np.ascontiguousarray(out)


LAST_RESULT = None
TRACE = False


def kernel(source, target, weight):
    from concourse.bass_utils import run_bass_kernel_spmd

    global LAST_RESULT
    in_maps = make_host_inputs(
        np.asarray(source), np.asarray(target), np.asarray(weight)
    )
    nc = _get_nc()
    res = run_bass_kernel_spmd(
        nc, in_maps, core_ids=list(range(N_CORES)), trace=TRACE
    )
    LAST_RESULT = res
    return unpack_output(res.results)

